# revision 25
# baseline (speedup 1.0000x reference)
"""CSWin block (B=8,H=W=56,C=256) on 8 trn2 NeuronCores, data-parallel over batch.

v2 layout strategy (per core, one image of 3136 tokens):
  - Residual stream token-major [112, 28, 256] fp32; LN stats token-major,
    LN output channel-major fp8 [128, 2(kch), T] feeding DoubleRow matmuls.
  - fp8e4 DoubleRow (0.5 cyc/output-row, 256-deep contraction per instruction)
    for QKV, fc1, fc2, proj. Biases for proj/fc2 enter PSUM via K=1 ones
    matmuls. b_fc1 rides the GELU activation per-partition.
  - Attention O is computed TRANSPOSED (window q-tokens on partitions) via
    fp8 DoubleRow over k-chunk pairs; a ones-column in V^T produces the
    softmax denominator as a per-partition column -> normalize is a
    per-partition reciprocal + stride-0-broadcast multiply, then one small
    PE transpose per q-chunk returns to channel-major for proj.
  - Branch-0 q/k/v stored W-major so its stripe windows are contiguous;
    PSUM evacuations scatter between H-major and W-major orders.
  - LePE depthwise conv stays on the PE as 9 diag-matmul taps (bf16).
  - exp on the Act engine is the wall (~95us); everything else is spread
    across DVE/Pool/PE underneath it.
"""

import sys

sys.path.insert(0, "/opt/trn_rl_repo")

import numpy as np
from contextlib import ExitStack

import concourse.bacc as bacc
import concourse.tile as tile
import concourse.mybir as mybir
from concourse.bass_utils import run_bass_kernel_spmd
from concourse.masks import make_identity

F32 = mybir.dt.float32
BF16 = mybir.dt.bfloat16
F8 = mybir.dt.float8e4
AF = mybir.ActivationFunctionType
OP = mybir.AluOpType
PM = mybir.MatmulPerfMode

B, HH, WW, C = 8, 56, 56, 256
T = HH * WW              # 3136 tokens
NW = 8                   # windows per branch
L = 392                  # tokens per window
NH = 4                   # heads per branch
HD = 32                  # head dim
TK = 112                 # token chunk for token-major phases
NTOK = T // TK           # 28
MCS = [112, 112, 112, 56]  # window k/q chunk sizes (112*3 + 56 = 392)
MCO = [0, 112, 224, 336]   # their offsets
EPS = 1e-5
SM_SCALE = float(HD) ** -0.5

_CACHE = {}


def _build():
    nc = bacc.Bacc("TRN2", target_bir_lowering=False, debug=False,
                   enable_asserts=False, num_devices=8)

    x_d = nc.dram_tensor("x", [T, C], F32, kind="ExternalInput").ap()
    out_d = nc.dram_tensor("out", [T, C], F32, kind="ExternalOutput").ap()
    wqkv_d = nc.dram_tensor("w_qkv8", [128, 2, 3 * C], F8, kind="ExternalInput").ap()
    wproj_d = nc.dram_tensor("w_proj8", [128, 2, C], F8, kind="ExternalInput").ap()
    g1_d = nc.dram_tensor("gamma1", [C], F32, kind="ExternalInput").ap()
    be1_d = nc.dram_tensor("beta1", [C], F32, kind="ExternalInput").ap()
    g2_d = nc.dram_tensor("gamma2", [C], F32, kind="ExternalInput").ap()
    be2_d = nc.dram_tensor("beta2", [C], F32, kind="ExternalInput").ap()
    wfc1_d = nc.dram_tensor("w_fc18", [128, 2, 4 * C], F8, kind="ExternalInput").ap()
    bfc1_d = nc.dram_tensor("b_fc1", [4 * C], F32, kind="ExternalInput").ap()
    wfc2_d = nc.dram_tensor("w_fc28", [128, 8, C], F8, kind="ExternalInput").ap()
    lw_d = nc.dram_tensor("lwb16", [128, 2, 9], BF16, kind="ExternalInput").ap()
    brow_d = nc.dram_tensor("brow16", [1, 2, C], BF16, kind="ExternalInput").ap()
    lb_d = [nc.dram_tensor("lepe_b0", [128], F32, kind="ExternalInput").ap(),
            nc.dram_tensor("lepe_b1", [128], F32, kind="ExternalInput").ap()]

    with tile.TileContext(nc) as tc:
        with ExitStack() as ctx:
            _emit(nc, tc, ctx, locals())
    nc.compile()
    return nc


def _emit(nc, tc, ctx, d):
    x_d, out_d = d["x_d"], d["out_d"]
    lw_d, lb_d = d["lw_d"], d["lb_d"]

    pp = ctx.enter_context(tc.tile_pool(name="pp", bufs=1))
    psmall = ctx.enter_context(tc.tile_pool(name="psmall", bufs=1))

    # ---------------- persistent tensors ----------------
    xtok = pp.tile([TK, NTOK, C], F32, name="xtok")
    x2res = pp.tile([TK, NTOK, C], F32, name="x2res")
    wqkv8 = pp.tile([128, 2, 3 * C], F8, name="wqkv8")
    wproj8 = pp.tile([128, 2, C], F8, name="wproj8")
    wfc18 = pp.tile([128, 2, 4 * C], F8, name="wfc18")
    wfc28 = pp.tile([128, 8, C], F8, name="wfc28")
    lwb = pp.tile([128, 2, 9], BF16, name="lwb")
    ln2 = pp.tile([128, 2, T], F8, name="ln2")
    g1s = psmall.tile([128, 2], F32, name="g1s")
    b1s = psmall.tile([128, 2], F32, name="b1s")
    g2s = psmall.tile([128, 2], F32, name="g2s")
    b2s = psmall.tile([128, 2], F32, name="b2s")
    bfc1s = psmall.tile([128, 8], F32, name="bfc1s")
    lbs = psmall.tile([128, 2], F32, name="lbs")
    brow_sb = psmall.tile([1, 2, C], BF16, name="brow_sb")
    ones1 = psmall.tile([1, TK], BF16, name="ones1")
    id_b = psmall.tile([128, 128], BF16, name="id_b")
    mv1 = psmall.tile([TK, NTOK, 2], F32, name="mv1")
    rstd1 = psmall.tile([TK, NTOK], F32, name="rstd1")
    std1 = psmall.tile([TK, NTOK], F32, name="std1")
    mv2 = psmall.tile([TK, NTOK, 2], F32, name="mv2")
    rstd2 = psmall.tile([TK, NTOK], F32, name="rstd2")
    std2 = psmall.tile([TK, NTOK], F32, name="std2")
    eps_t = psmall.tile([TK, 1], F32, name="eps_t")
    nc.vector.memset(eps_t, EPS)
    nc.vector.memset(ones1, 1.0)

    # ---------------- phase A: loads ----------------
    with tc.tile_pool(name="stg", bufs=1) as stg:
        xsrc = x_d.rearrange("(i p) c -> p i c", p=TK)
        for j in range(7):
            eng = [nc.sync, nc.scalar, nc.gpsimd][j % 3]
            eng.dma_start(xtok[:, 4 * j:4 * j + 4, :], xsrc[:, 4 * j:4 * j + 4, :])
        nc.sync.dma_start(wqkv8, d["wqkv_d"])
        nc.scalar.dma_start(wproj8, d["wproj_d"])
        nc.gpsimd.dma_start(wfc18, d["wfc1_d"])
        nc.sync.dma_start(wfc28, d["wfc2_d"])
        nc.scalar.dma_start(lwb, d["lw_d"])
        nc.gpsimd.dma_start(brow_sb, d["brow_d"])
        for bi in range(2):
            nc.sync.dma_start(lbs[:, bi:bi + 1], lb_d[bi].unsqueeze(1))
        nc.sync.dma_start(g1s, d["g1_d"].rearrange("(a p) -> p a", p=128))
        nc.sync.dma_start(b1s, d["be1_d"].rearrange("(a p) -> p a", p=128))
        nc.scalar.dma_start(g2s, d["g2_d"].rearrange("(a p) -> p a", p=128))
        nc.scalar.dma_start(b2s, d["be2_d"].rearrange("(a p) -> p a", p=128))
        nc.gpsimd.dma_start(bfc1s, d["bfc1_d"].rearrange("(a p) -> p a", p=128))
        id_f = stg.tile([128, 128], F32, name="id_f")
        make_identity(nc, id_f)
        nc.vector.tensor_copy(id_b, id_f)

    # ---------------- LN (token-major) helper ----------------
    def layer_norm(src, mv, stdt, rstd, gs, bs, dst, lnp, tpp, apply_act,
                   stats=True):
        """src: [TK, NTOK, C] f32; dst: channel-major [128, 2, T] fp8 tile."""
        if stats:
            for i in range(NTOK):
                st = lnp.tile([TK, 6], F32, name="bnst", tag="bnst")
                nc.vector.bn_stats(st, src[:, i, :])
                nc.vector.bn_aggr(mv[:, i, :], st)
        nc.scalar.activation(stdt, mv[:, :, 1], AF.Sqrt, bias=eps_t)
        nc.vector.reciprocal(rstd, stdt)
        for g in range(7):
            lnt = []
            for j in range(4):
                i = 4 * g + j
                lt = lnp.tile([TK, C], BF16, name="lnt", tag="lnt")
                nc.vector.tensor_scalar(
                    out=lt, in0=src[:, i, :],
                    scalar1=mv[:, i, 0:1], scalar2=rstd[:, i:i + 1],
                    op0=OP.subtract, op1=OP.mult)
                lnt.append(lt)
            for c in range(2):
                tp = tpp.tile([128, 4 * TK], BF16, name="lntp", tag="lntp")
                for j in range(4):
                    nc.tensor.transpose(tp[:, TK * j:TK * (j + 1)],
                                        lnt[j][:, 128 * c:128 * (c + 1)],
                                        id_b[0:TK, 0:TK])
                if apply_act:
                    nc.scalar.activation(dst[:, c, 4 * TK * g:4 * TK * (g + 1)], tp,
                                         AF.Identity, bias=bs[:, c:c + 1],
                                         scale=gs[:, c:c + 1])
                else:
                    nc.vector.tensor_scalar(
                        out=dst[:, c, 4 * TK * g:4 * TK * (g + 1)], in0=tp,
                        scalar1=gs[:, c:c + 1], scalar2=bs[:, c:c + 1],
                        op0=OP.mult, op1=OP.add)

    # ---------------- attention-lifetime tensors ----------------
    actx = ExitStack()
    attn_pool = actx.enter_context(tc.tile_pool(name="attn_pool", bufs=1))
    ln1 = attn_pool.tile([128, 2, T], F8, name="ln1")
    qc = [attn_pool.tile([128, T], BF16, name=f"qc{b}") for b in range(2)]
    kc = [attn_pool.tile([128, T], BF16, name=f"kc{b}") for b in range(2)]
    vc = [attn_pool.tile([128, T], BF16, name=f"vc{b}") for b in range(2)]
    # vtb: [k-token, w, m, h, 36] fp8; col 32 of each 36-block is the ones col
    vtb = [attn_pool.tile([TK, NW, 4, NH, 36], F8, name=f"vtb{b}") for b in range(2)]
    lepe_sb = [attn_pool.tile([128, T], BF16, name=f"lepe{b}") for b in range(2)]
    att = attn_pool.tile([128, 2, T], F8, name="att")
    # double-buffered exp output: [k-token, m, h, q]
    etb = [attn_pool.tile([TK, 4, NH, L], F8, name=f"et{z}") for z in range(2)]
    dg = [attn_pool.tile([128, 9, 128], BF16, name=f"dg{b}") for b in range(2)]

    # zero-init (Pool, overlapped with loads): vtb + et m3 tail rows
    for b in range(2):
        nc.gpsimd.memset(vtb[b], 0.0)
    for z in range(2):
        nc.gpsimd.memset(etb[z][:, 3, :, :], 0.0)
    # LePE diag matrices (Pool, overlapped with loads)
    for bi in range(2):
        for tx in (-1, 0, 1):
            for ty in (-1, 0, 1):
                wi = ((ty + 1) * 3 + (tx + 1)) if bi == 0 else ((tx + 1) * 3 + (ty + 1))
                nc.gpsimd.affine_select(
                    out=dg[bi][:, wi, :],
                    in_=lwb[:, bi, wi:wi + 1].broadcast_to([128, 128]),
                    compare_op=OP.is_equal, fill=0.0, base=0,
                    pattern=[[-1, 128]], channel_multiplier=1)

    # Optional in-NEFF repetition loop for wall-clock timing (BASS_KERNEL_ITERS>1)
    import os as _os
    _iters = int(_os.environ.get("BASS_KERNEL_ITERS", "1"))
    loop_cm = tc.For_i(0, _iters, 1) if _iters > 1 else None
    if loop_cm is not None:
        ctx.enter_context(loop_cm)

    # ---------------- phase B+C: LN1 pipelined with QKV ----------------
    qc0w = qc[0].rearrange("p (w h) -> p h w", w=WW)
    kc0w = kc[0].rearrange("p (w h) -> p h w", w=WW)
    vc0w = vc[0].rearrange("p (w h) -> p h w", w=WW)
    chunks = [("k0", 0, 256, nc.scalar), ("k1", 1, 384, nc.vector),
              ("q0", 0, 0, nc.scalar), ("q1", 1, 128, nc.scalar),
              ("v0", 0, 512, nc.scalar), ("v1", 1, 640, nc.vector)]
    wmaj = {"q0": qc0w, "k0": kc0w, "v0": vc0w}
    hmaj = {"q1": qc[1], "k1": kc[1], "v1": vc[1]}

    with tc.tile_pool(name="lnp1", bufs=8) as lnp, \
         tc.tile_pool(name="tpp1", bufs=2, space="PSUM") as tpp, \
         tc.tile_pool(name="qkvps", bufs=3, space="PSUM") as qkvps:

        def emit_qkv_t2(t2):
            for nm, bi, c0, eng in chunks:
                pt = qkvps.tile([128, 2, 512], F32, name="qkvt", tag="qkvt")
                for half in range(2):
                    t = 2 * t2 + half
                    nc.tensor.matmul(pt[:, half, 0:L],
                                     wqkv8[:, :, c0:c0 + 128],
                                     ln1[:, :, L * t:L * (t + 1)],
                                     start=True, stop=True,
                                     perf_mode=PM.DoubleRow)
                t0 = 2 * t2
                if nm in wmaj:
                    dstw = wmaj[nm][:, 7 * t0:7 * t0 + 14, :].rearrange(
                        "p (a h) w -> p a h w", a=2)
                    s = pt[:, :, 0:L].rearrange("p a (x y) -> p a x y", x=7)
                    if eng is nc.scalar:
                        eng.copy(dstw, s)
                    else:
                        eng.tensor_copy(dstw, s)
                else:
                    dsth = hmaj[nm][:, L * t0:L * (t0 + 2)].rearrange(
                        "p (a x) -> p a x", a=2)
                    if eng is nc.scalar:
                        eng.copy(dsth, pt[:, :, 0:L])
                    else:
                        eng.tensor_copy(dsth, pt[:, :, 0:L])

        qkv_done = 0
        for g in range(7):
            i0 = 4 * g
            for i in range(i0, i0 + 4):
                st = lnp.tile([TK, 6], F32, name="bnst", tag="bnst")
                nc.vector.bn_stats(st, xtok[:, i, :])
                nc.vector.bn_aggr(mv1[:, i, :], st)
            nc.scalar.activation(std1[:, i0:i0 + 4], mv1[:, i0:i0 + 4, 1],
                                 AF.Sqrt, bias=eps_t)
            nc.vector.reciprocal(rstd1[:, i0:i0 + 4], std1[:, i0:i0 + 4])
            lnt = []
            for j in range(4):
                i = i0 + j
                lt = lnp.tile([TK, C], BF16, name="lnt", tag="lnt")
                nc.vector.tensor_scalar(
                    out=lt, in0=xtok[:, i, :],
                    scalar1=mv1[:, i, 0:1], scalar2=rstd1[:, i:i + 1],
                    op0=OP.subtract, op1=OP.mult)
                lnt.append(lt)
            for c in range(2):
                tp = tpp.tile([128, 4 * TK], BF16, name="lntp", tag="lntp")
                for j in range(4):
                    nc.tensor.transpose(tp[:, TK * j:TK * (j + 1)],
                                        lnt[j][:, 128 * c:128 * (c + 1)],
                                        id_b[0:TK, 0:TK])
                nc.scalar.activation(ln1[:, c, 4 * TK * g:4 * TK * (g + 1)], tp,
                                     AF.Identity, bias=b1s[:, c:c + 1],
                                     scale=g1s[:, c:c + 1])
            while qkv_done < 4 and 784 * (qkv_done + 1) <= 448 * (g + 1):
                emit_qkv_t2(qkv_done)
                qkv_done += 1

    # ---------------- phase D: windowed attention ----------------
    # PSUM: sp tag = [128, 2, 512] f32 (2 banks) x2 bufs; aux tag = 1-bank
    # x4 bufs holding lp / otA / otB / ptp / prt. exp runs per (m, head-pair)
    # so S double-buffering never waits on the window tail.
    att0w = att[:, 0, :].rearrange("p (w h) -> p h w", w=WW)
    taps = [(0, 0)] + [(tx, ty) for tx in (-1, 0, 1) for ty in (-1, 0, 1)
                       if (tx, ty) != (0, 0)]

    def emit_proj(i2, pool):
        pt = pool.tile([TK, 2, C], F32, name="prt", tag="aux", bufs=3)
        for half in range(2):
            i = 2 * i2 + half
            nc.tensor.matmul(pt[:, half, :], ones1, brow_sb[:, 0, :],
                             start=(half == 0), stop=False,
                             skip_group_check=True)
            nc.tensor.matmul(pt[:, half, :],
                             att[:, :, TK * i:TK * (i + 1)],
                             wproj8,
                             start=False, stop=(half == 1),
                             perf_mode=PM.DoubleRow, skip_group_check=True)
        nc.vector.scalar_tensor_tensor(
            out=x2res[:, 2 * i2:2 * i2 + 2, :], in0=pt, scalar=1.0,
            in1=xtok[:, 2 * i2:2 * i2 + 2, :], op0=OP.mult, op1=OP.add)
        # LN2 stats for these two token tiles (DVE, no PSUM)
        for i in (2 * i2, 2 * i2 + 1):
            st = onp.tile([TK, 6], F32, name="bnst2", tag="bnst2")
            nc.vector.bn_stats(st, x2res[:, i, :])
            nc.vector.bn_aggr(mv2[:, i, :], st)

    proj_done = 0
    with tc.tile_pool(name="dps", bufs=2, space="PSUM") as dps, \
         tc.tile_pool(name="onp", bufs=3) as onp, \
         tc.tile_pool(name="rqp", bufs=3) as rqp:
        def emit_vtb(bj, wj):
            vt = dps.tile([TK, 4, 128], BF16, name="vtt", tag="aux", bufs=3)
            for m in range(4):
                mc = MCS[m]
                nc.tensor.transpose(
                    vt[0:mc, m, :],
                    vc[bj][:, L * wj + MCO[m]:L * wj + MCO[m] + mc],
                    id_b)
            vtv = vt.rearrange("p m (h e) -> p m h e", h=NH)
            nc.vector.tensor_copy(vtb[bj][:, wj, 0:3, :, 0:32], vtv[:, 0:3])
            nc.vector.tensor_copy(vtb[bj][0:56, wj, 3, :, 0:32], vtv[0:56, 3])
            nc.vector.memset(vtb[bj][:, wj, :, :, 32:33], 1.0)

        def emit_sexp(idx):
            bi, w = idx // NW, idx % NW
            et = etb[idx % 2]
            for m in range(4):
                mc = MCS[m]
                for hp in range(2):
                    sp = dps.tile([128, 2, 512], F32, name="spst", tag="sp")
                    for hh in range(2):
                        h = 2 * hp + hh
                        nc.tensor.matmul(
                            sp[0:mc, hh, 0:L],
                            kc[bi][32 * h:32 * (h + 1),
                                   L * w + MCO[m]:L * w + MCO[m] + mc],
                            qc[bi][32 * h:32 * (h + 1), L * w:L * (w + 1)],
                            start=True, stop=True, tile_position=(32 * h, 0))
                    nc.scalar.activation(
                        et[0:mc, m, 2 * hp:2 * hp + 2, :],
                        sp[0:mc, :, 0:L],
                        AF.Exp, scale=SM_SCALE)

        def emit_tail(idx):
            nonlocal proj_done
            bi, w = idx // NW, idx % NW
            et = etb[idx % 2]
            vcv = vc[bi][:, L * w:L * (w + 1)].rearrange("p (x y) -> p x y", x=7)
            if bi == 0:
                emit_vtb(0, w)
                emit_vtb(1, w)
            # LePE taps
            lp = dps.tile([128, 512], F32, name="lpt", tag="aux", bufs=3)
            lpv = lp[:, 0:L].rearrange("p (x y) -> p x y", x=7)
            for ti, (tx, ty) in enumerate(taps):
                wi = ((ty + 1) * 3 + (tx + 1)) if bi == 0 else ((tx + 1) * 3 + (ty + 1))
                xo0, xo1 = max(0, -tx), 7 - max(0, tx)
                yo0, yo1 = max(0, -ty), HH - max(0, ty)
                nc.tensor.matmul(
                    lpv[:, xo0:xo1, yo0:yo1], dg[bi][:, wi, :],
                    vcv[:, xo0 + tx:xo1 + tx, yo0 + ty:yo1 + ty],
                    start=(ti == 0), stop=(ti == 8))
            nc.vector.tensor_scalar(
                out=lepe_sb[bi][:, L * w:L * (w + 1)], in0=lp[:, 0:L],
                scalar1=lbs[:, bi:bi + 1], scalar2=None, op0=OP.add)
            # O^T via fp8 DoubleRow
            ots = []
            for qp in range(2):
                ot = dps.tile([TK, 2, NH, 34], F32, name="otst", tag="aux",
                              bufs=3)
                ots.append(ot)
                first = True
                for qq in range(2):
                    q = 2 * qp + qq
                    qn = MCS[q]
                    for j in range(2):
                        for h in range(NH):
                            nc.tensor.matmul(
                                ot[0:qn, qq, h, 0:33],
                                et[:, 2 * j:2 * j + 2, h, MCO[q]:MCO[q] + qn],
                                vtb[bi][:, w, 2 * j:2 * j + 2, h, 0:33],
                                start=first,
                                stop=(qq == 1 and j == 1 and h == NH - 1),
                                perf_mode=PM.DoubleRow, skip_group_check=True)
                            first = False
            ptp = dps.tile([128, NH, TK], BF16, name="ptp", tag="aux", bufs=3)
            for q in range(4):
                qn, qo = MCS[q], MCO[q]
                ot = ots[q // 2]
                rq = rqp.tile([TK, NH], F32, name="rq", tag="rq")
                nc.vector.reciprocal(rq[0:qn], ot[0:qn, q % 2, :, 32])
                on = onp.tile([TK, NH * 32], BF16, name="on", tag="on")
                onv = on.rearrange("p (a b) -> p a b", a=NH)
                nc.vector.tensor_tensor(
                    out=onv[0:qn], in0=ot[0:qn, q % 2, :, 0:32],
                    in1=rq[0:qn].unsqueeze(2).broadcast_to([qn, NH, 32]),
                    op=OP.mult)
                nc.tensor.transpose(ptp[:, q, 0:qn], on[0:qn, :],
                                    id_b[0:qn, 0:qn])
                lep = lepe_sb[bi][:, L * w + qo:L * w + qo + qn]
                if bi == 0:
                    dst = att0w[:, :, 7 * w + qo // 56:7 * w + (qo + qn) // 56]
                    nc.vector.tensor_tensor(
                        out=dst,
                        in0=ptp[:, q, 0:qn].rearrange("p (a b) -> p b a", b=HH),
                        in1=lep.rearrange("p (a b) -> p b a", b=HH),
                        op=OP.add)
                else:
                    nc.vector.tensor_tensor(
                        out=att[:, 1, L * w + qo:L * w + qo + qn],
                        in0=ptp[:, q, 0:qn], in1=lep, op=OP.add)
            if bi == 1:
                while 224 * (proj_done + 1) <= 392 * (w + 1):
                    emit_proj(proj_done, dps)
                    proj_done += 1

        emit_sexp(0)
        for idx in range(2 * NW):
            if idx + 1 < 2 * NW:
                emit_sexp(idx + 1)
            emit_tail(idx)
    # ---------------- phase E: proj + residual (token-major out) ----------------
    actx.close()

    # ---------------- phase F: LN2 + MLP ----------------
    with tc.tile_pool(name="mlp", bufs=1) as mlp:
        h_sb = mlp.tile([128, 8, T], F8, name="h_sb")
        with tc.tile_pool(name="lnp2", bufs=8) as lnp2, \
             tc.tile_pool(name="tpp2", bufs=2, space="PSUM") as tpp2:
            for gs, ge in ((0, 6), (6, 7)):
                i0 = 4 * gs
                i1 = 4 * ge
                nc.scalar.activation(std2[:, i0:i1], mv2[:, i0:i1, 1],
                                     AF.Sqrt, bias=eps_t)
                nc.vector.reciprocal(rstd2[:, i0:i1], std2[:, i0:i1])
                for g in range(gs, ge):
                    lnt = []
                    for j in range(4):
                        i = 4 * g + j
                        lt = lnp2.tile([TK, C], BF16, name="lnt2", tag="lnt2")
                        nc.vector.tensor_scalar(
                            out=lt, in0=x2res[:, i, :],
                            scalar1=mv2[:, i, 0:1], scalar2=rstd2[:, i:i + 1],
                            op0=OP.subtract, op1=OP.mult)
                        lnt.append(lt)
                    for c in range(2):
                        tp = tpp2.tile([128, 4 * TK], BF16, name="lntp2",
                                       tag="lntp2")
                        for j in range(4):
                            nc.tensor.transpose(tp[:, TK * j:TK * (j + 1)],
                                                lnt[j][:, 128 * c:128 * (c + 1)],
                                                id_b[0:TK, 0:TK])
                        nc.scalar.activation(
                            ln2[:, c, 4 * TK * g:4 * TK * (g + 1)], tp,
                            AF.Identity, bias=b2s[:, c:c + 1],
                            scale=g2s[:, c:c + 1])
        with tc.tile_pool(name="f1ps", bufs=2, space="PSUM") as f1ps, \
             tc.tile_pool(name="f2ps", bufs=2, space="PSUM") as f2ps, \
             tc.tile_pool(name="otp", bufs=4) as otp:
            fc2_done = 0
            for tp2 in range(NW // 2):
                tparts = [(2 * tp2, 2)] if tp2 < 3 else [(6, 1), (7, 1)]
                for tb, tn in tparts:
                    for m8 in range(8):
                        pt = f1ps.tile([128, 2, 512], F32, name="f1t", tag="f1t")
                        for half in range(tn):
                            t = tb + half
                            nc.tensor.matmul(pt[:, half, 0:L],
                                             wfc18[:, :, 128 * m8:128 * (m8 + 1)],
                                             ln2[:, :, L * t:L * (t + 1)],
                                             start=True, stop=True,
                                             perf_mode=PM.DoubleRow)
                        nc.scalar.activation(
                            h_sb[:, m8, L * tb:L * (tb + tn)].rearrange(
                                "p (a x) -> p a x", a=tn),
                            pt[:, 0:tn, 0:L],
                            AF.Gelu, bias=bfc1s[:, m8:m8 + 1])
                    while 224 * (fc2_done + 1) <= 392 * (tb + tn):
                        i2 = fc2_done
                        pt = f2ps.tile([TK, 2, C], F32, name="f2t", tag="f2t")
                        for half in range(2):
                            i = 2 * i2 + half
                            nc.tensor.matmul(pt[:, half, :], ones1,
                                             brow_sb[:, 1, :],
                                             start=(half == 0), stop=False,
                                             skip_group_check=True)
                            for j in range(4):
                                nc.tensor.matmul(
                                    pt[:, half, :],
                                    h_sb[:, 2 * j:2 * j + 2, TK * i:TK * (i + 1)],
                                    wfc28[:, 2 * j:2 * j + 2, :],
                                    start=False,
                                    stop=(half == 1 and j == 3),
                                    perf_mode=PM.DoubleRow,
                                    skip_group_check=True)
                        ot = otp.tile([TK, 2, C], F32, name="ot", tag="ot")
                        nc.vector.scalar_tensor_tensor(
                            out=ot, in0=pt, scalar=1.0,
                            in1=x2res[:, 2 * i2:2 * i2 + 2, :],
                            op0=OP.mult, op1=OP.add)
                        eng = nc.sync if i2 % 2 == 0 else nc.scalar
                        eng.dma_start(
                            out_d[2 * TK * i2:2 * TK * (i2 + 1), :].rearrange(
                                "(a p) c -> p a c", p=TK),
                            ot)
                        fc2_done += 1
            if False:
                while 224 * (fc2_done + 1) <= 784 * (tp2 + 1):
                    i2 = fc2_done
                    pt = f2ps.tile([TK, 2, C], F32, name="f2t", tag="f2t")
                    for half in range(2):
                        i = 2 * i2 + half
                        nc.tensor.matmul(pt[:, half, :], ones1, brow_sb[:, 1, :],
                                         start=(half == 0), stop=False,
                                         skip_group_check=True)
                        for j in range(4):
                            nc.tensor.matmul(pt[:, half, :],
                                             h_sb[:, 2 * j:2 * j + 2, TK * i:TK * (i + 1)],
                                             wfc28[:, 2 * j:2 * j + 2, :],
                                             start=False,
                                             stop=(half == 1 and j == 3),
                                             perf_mode=PM.DoubleRow,
                                             skip_group_check=True)
                    ot = otp.tile([TK, 2, C], F32, name="ot", tag="ot")
                    nc.vector.scalar_tensor_tensor(
                        out=ot, in0=pt, scalar=1.0,
                        in1=x2res[:, 2 * i2:2 * i2 + 2, :],
                        op0=OP.mult, op1=OP.add)
                    eng = nc.sync if i2 % 2 == 0 else nc.scalar
                    eng.dma_start(
                        out_d[2 * TK * i2:2 * TK * (i2 + 1), :].rearrange(
                            "(a p) c -> p a c", p=TK),
                        ot)
                    fc2_done += 1


def kernel(**inputs):
    if "nc" not in _CACHE:
        _CACHE["nc"] = _build()
    nc = _CACHE["nc"]

    import ml_dtypes
    FP8 = ml_dtypes.float8_e4m3
    x = np.asarray(inputs["x"], dtype=np.float32)          # [8, 56, 56, 256]

    def pack8(w, parts):
        w = np.asarray(w, np.float32)
        return np.ascontiguousarray(
            w.reshape(parts, 128, w.shape[1]).transpose(1, 0, 2)).astype(FP8)

    lw = np.stack([np.asarray(inputs["lepe_w0"], np.float32).reshape(128, 9),
                   np.asarray(inputs["lepe_w1"], np.float32).reshape(128, 9)],
                  axis=1)
    brow = np.stack([np.asarray(inputs["b_proj"], np.float32),
                     np.asarray(inputs["b_fc2"], np.float32)])[None]
    base = {
        "w_qkv8": pack8(inputs["w_qkv"], 2),
        "w_proj8": pack8(inputs["w_proj"], 2),
        "gamma1": np.asarray(inputs["gamma1"], np.float32),
        "beta1": np.asarray(inputs["beta1"], np.float32),
        "gamma2": np.asarray(inputs["gamma2"], np.float32),
        "beta2": np.asarray(inputs["beta2"], np.float32),
        "w_fc18": pack8(inputs["w_fc1"], 2),
        "b_fc1": np.asarray(inputs["b_fc1"], np.float32),
        "w_fc28": pack8(inputs["w_fc2"], 8),
        "lwb16": lw.astype(ml_dtypes.bfloat16),
        "brow16": brow.astype(ml_dtypes.bfloat16),
        "lepe_b0": np.asarray(inputs["lepe_b0"], np.float32),
        "lepe_b1": np.asarray(inputs["lepe_b1"], np.float32),
    }
    in_maps = [{**base, "x": np.ascontiguousarray(x[i].reshape(T, C))}
               for i in range(B)]
    import os
    trace = bool(int(os.environ.get("BASS_KERNEL_TRACE", "0")))
    res = run_bass_kernel_spmd(nc, in_maps, core_ids=list(range(B)), trace=trace)
    _CACHE["last_results"] = res
    out = np.stack([res.results[i]["out"] for i in range(B)])
    return out.reshape(B, HH, WW, C)


if __name__ == "__main__":
    rng = np.random.default_rng(0)
    ins = {
        "x": rng.standard_normal((B, HH, WW, C), dtype=np.float32),
        "gamma1": np.ones(C, np.float32), "beta1": np.zeros(C, np.float32),
        "w_qkv": rng.standard_normal((C, 3 * C), dtype=np.float32) * 0.02,
        "lepe_w0": rng.standard_normal((128, 1, 3, 3), dtype=np.float32) * 0.02,
        "lepe_b0": np.zeros(128, np.float32),
        "lepe_w1": rng.standard_normal((128, 1, 3, 3), dtype=np.float32) * 0.02,
        "lepe_b1": np.zeros(128, np.float32),
        "w_proj": rng.standard_normal((C, C), dtype=np.float32) * 0.02,
        "b_proj": np.zeros(C, np.float32),
        "gamma2": np.ones(C, np.float32), "beta2": np.zeros(C, np.float32),
        "w_fc1": rng.standard_normal((C, 4 * C), dtype=np.float32) * 0.02,
        "b_fc1": np.zeros(4 * C, np.float32),
        "w_fc2": rng.standard_normal((4 * C, C), dtype=np.float32) * 0.02,
        "b_fc2": np.zeros(C, np.float32),
    }
    o = kernel(**ins)
    print("ran:", o.shape, o.dtype, float(np.abs(o).max()))


# revision 26
# speedup vs baseline: 1.0106x; 1.0106x over previous
"""CSWin block (B=8,H=W=56,C=256) on 8 trn2 NeuronCores, data-parallel over batch.

v2 layout strategy (per core, one image of 3136 tokens):
  - Residual stream token-major [112, 28, 256] fp32; LN stats token-major,
    LN output channel-major fp8 [128, 2(kch), T] feeding DoubleRow matmuls.
  - fp8e4 DoubleRow (0.5 cyc/output-row, 256-deep contraction per instruction)
    for QKV, fc1, fc2, proj. Biases for proj/fc2 enter PSUM via K=1 ones
    matmuls. b_fc1 rides the GELU activation per-partition.
  - Attention O is computed TRANSPOSED (window q-tokens on partitions) via
    fp8 DoubleRow over k-chunk pairs; a ones-column in V^T produces the
    softmax denominator as a per-partition column -> normalize is a
    per-partition reciprocal + stride-0-broadcast multiply, then one small
    PE transpose per q-chunk returns to channel-major for proj.
  - Branch-0 q/k/v stored W-major so its stripe windows are contiguous;
    PSUM evacuations scatter between H-major and W-major orders.
  - LePE depthwise conv stays on the PE as 9 diag-matmul taps (bf16).
  - exp on the Act engine is the wall (~95us); everything else is spread
    across DVE/Pool/PE underneath it.
"""

import sys

sys.path.insert(0, "/opt/trn_rl_repo")

import numpy as np
from contextlib import ExitStack

import concourse.bacc as bacc
import concourse.tile as tile
import concourse.mybir as mybir
from concourse.bass_utils import run_bass_kernel_spmd
from concourse.masks import make_identity

F32 = mybir.dt.float32
BF16 = mybir.dt.bfloat16
F8 = mybir.dt.float8e4
AF = mybir.ActivationFunctionType
OP = mybir.AluOpType
PM = mybir.MatmulPerfMode

B, HH, WW, C = 8, 56, 56, 256
T = HH * WW              # 3136 tokens
NW = 8                   # windows per branch
L = 392                  # tokens per window
NH = 4                   # heads per branch
HD = 32                  # head dim
TK = 112                 # token chunk for token-major phases
NTOK = T // TK           # 28
MCS = [112, 112, 112, 56]  # window k/q chunk sizes (112*3 + 56 = 392)
MCO = [0, 112, 224, 336]   # their offsets
EPS = 1e-5
SM_SCALE = float(HD) ** -0.5

_CACHE = {}


def _build():
    nc = bacc.Bacc("TRN2", target_bir_lowering=False, debug=False,
                   enable_asserts=False, num_devices=8)

    x_d = nc.dram_tensor("x", [T, C], F32, kind="ExternalInput").ap()
    out_d = nc.dram_tensor("out", [T, C], F32, kind="ExternalOutput").ap()
    wqkv_d = nc.dram_tensor("w_qkv8", [128, 2, 3 * C], F8, kind="ExternalInput").ap()
    wproj_d = nc.dram_tensor("w_proj8", [128, 2, C], F8, kind="ExternalInput").ap()
    g1_d = nc.dram_tensor("gamma1", [C], F32, kind="ExternalInput").ap()
    be1_d = nc.dram_tensor("beta1", [C], F32, kind="ExternalInput").ap()
    g2_d = nc.dram_tensor("gamma2", [C], F32, kind="ExternalInput").ap()
    be2_d = nc.dram_tensor("beta2", [C], F32, kind="ExternalInput").ap()
    wfc1_d = nc.dram_tensor("w_fc18", [128, 2, 4 * C], F8, kind="ExternalInput").ap()
    bfc1_d = nc.dram_tensor("b_fc1", [4 * C], F32, kind="ExternalInput").ap()
    wfc2_d = nc.dram_tensor("w_fc28", [128, 8, C], F8, kind="ExternalInput").ap()
    lw_d = nc.dram_tensor("lwb16", [128, 2, 9], BF16, kind="ExternalInput").ap()
    brow_d = nc.dram_tensor("brow16", [1, 2, C], BF16, kind="ExternalInput").ap()
    lb_d = [nc.dram_tensor("lepe_b0", [128], F32, kind="ExternalInput").ap(),
            nc.dram_tensor("lepe_b1", [128], F32, kind="ExternalInput").ap()]

    with tile.TileContext(nc) as tc:
        with ExitStack() as ctx:
            _emit(nc, tc, ctx, locals())
    nc.compile()
    return nc


def _emit(nc, tc, ctx, d):
    x_d, out_d = d["x_d"], d["out_d"]
    lw_d, lb_d = d["lw_d"], d["lb_d"]

    pp = ctx.enter_context(tc.tile_pool(name="pp", bufs=1))
    psmall = ctx.enter_context(tc.tile_pool(name="psmall", bufs=1))

    # ---------------- persistent tensors ----------------
    xtok = pp.tile([TK, NTOK, C], F32, name="xtok")
    x2res = pp.tile([TK, NTOK, C], F32, name="x2res")
    wqkv8 = pp.tile([128, 2, 3 * C], F8, name="wqkv8")
    wproj8 = pp.tile([128, 2, C], F8, name="wproj8")
    wfc18 = pp.tile([128, 2, 4 * C], F8, name="wfc18")
    wfc28 = pp.tile([128, 8, C], F8, name="wfc28")
    lwb = pp.tile([128, 2, 9], BF16, name="lwb")
    ln2 = pp.tile([128, 2, T], F8, name="ln2")
    g1s = psmall.tile([128, 2], F32, name="g1s")
    b1s = psmall.tile([128, 2], F32, name="b1s")
    g2s = psmall.tile([128, 2], F32, name="g2s")
    b2s = psmall.tile([128, 2], F32, name="b2s")
    bfc1s = psmall.tile([128, 8], F32, name="bfc1s")
    lbs = psmall.tile([128, 2], F32, name="lbs")
    brow_sb = psmall.tile([1, 2, C], BF16, name="brow_sb")
    ones1 = psmall.tile([1, TK], BF16, name="ones1")
    id_b = psmall.tile([128, 128], BF16, name="id_b")
    mv1 = psmall.tile([TK, NTOK, 2], F32, name="mv1")
    rstd1 = psmall.tile([TK, NTOK], F32, name="rstd1")
    std1 = psmall.tile([TK, NTOK], F32, name="std1")
    mv2 = psmall.tile([TK, NTOK, 2], F32, name="mv2")
    rstd2 = psmall.tile([TK, NTOK], F32, name="rstd2")
    std2 = psmall.tile([TK, NTOK], F32, name="std2")
    eps_t = psmall.tile([TK, 1], F32, name="eps_t")
    nc.vector.memset(eps_t, EPS)
    nc.vector.memset(ones1, 1.0)

    # ---------------- phase A: loads ----------------
    with tc.tile_pool(name="stg", bufs=1) as stg:
        xsrc = x_d.rearrange("(i p) c -> p i c", p=TK)
        for j in range(7):
            eng = [nc.sync, nc.scalar, nc.gpsimd][j % 3]
            eng.dma_start(xtok[:, 4 * j:4 * j + 4, :], xsrc[:, 4 * j:4 * j + 4, :])
        nc.sync.dma_start(wqkv8, d["wqkv_d"])
        nc.scalar.dma_start(wproj8, d["wproj_d"])
        nc.gpsimd.dma_start(wfc18, d["wfc1_d"])
        nc.sync.dma_start(wfc28, d["wfc2_d"])
        nc.scalar.dma_start(lwb, d["lw_d"])
        nc.gpsimd.dma_start(brow_sb, d["brow_d"])
        for bi in range(2):
            nc.sync.dma_start(lbs[:, bi:bi + 1], lb_d[bi].unsqueeze(1))
        nc.sync.dma_start(g1s, d["g1_d"].rearrange("(a p) -> p a", p=128))
        nc.sync.dma_start(b1s, d["be1_d"].rearrange("(a p) -> p a", p=128))
        nc.scalar.dma_start(g2s, d["g2_d"].rearrange("(a p) -> p a", p=128))
        nc.scalar.dma_start(b2s, d["be2_d"].rearrange("(a p) -> p a", p=128))
        nc.gpsimd.dma_start(bfc1s, d["bfc1_d"].rearrange("(a p) -> p a", p=128))
        id_f = stg.tile([128, 128], F32, name="id_f")
        make_identity(nc, id_f)
        nc.vector.tensor_copy(id_b, id_f)

    # ---------------- LN (token-major) helper ----------------
    def layer_norm(src, mv, stdt, rstd, gs, bs, dst, lnp, tpp, apply_act,
                   stats=True):
        """src: [TK, NTOK, C] f32; dst: channel-major [128, 2, T] fp8 tile."""
        if stats:
            for i in range(NTOK):
                st = lnp.tile([TK, 6], F32, name="bnst", tag="bnst")
                nc.vector.bn_stats(st, src[:, i, :])
                nc.vector.bn_aggr(mv[:, i, :], st)
        nc.scalar.activation(stdt, mv[:, :, 1], AF.Sqrt, bias=eps_t)
        nc.vector.reciprocal(rstd, stdt)
        for g in range(7):
            lnt = []
            for j in range(4):
                i = 4 * g + j
                lt = lnp.tile([TK, C], BF16, name="lnt", tag="lnt")
                nc.vector.tensor_scalar(
                    out=lt, in0=src[:, i, :],
                    scalar1=mv[:, i, 0:1], scalar2=rstd[:, i:i + 1],
                    op0=OP.subtract, op1=OP.mult)
                lnt.append(lt)
            for c in range(2):
                tp = tpp.tile([128, 4 * TK], BF16, name="lntp", tag="lntp")
                for j in range(4):
                    nc.tensor.transpose(tp[:, TK * j:TK * (j + 1)],
                                        lnt[j][:, 128 * c:128 * (c + 1)],
                                        id_b[0:TK, 0:TK])
                if apply_act:
                    nc.scalar.activation(dst[:, c, 4 * TK * g:4 * TK * (g + 1)], tp,
                                         AF.Identity, bias=bs[:, c:c + 1],
                                         scale=gs[:, c:c + 1])
                else:
                    nc.vector.tensor_scalar(
                        out=dst[:, c, 4 * TK * g:4 * TK * (g + 1)], in0=tp,
                        scalar1=gs[:, c:c + 1], scalar2=bs[:, c:c + 1],
                        op0=OP.mult, op1=OP.add)

    # ---------------- attention-lifetime tensors ----------------
    actx = ExitStack()
    attn_pool = actx.enter_context(tc.tile_pool(name="attn_pool", bufs=1))
    ln1 = attn_pool.tile([128, 2, T], F8, name="ln1")
    qc = [attn_pool.tile([128, T], BF16, name=f"qc{b}") for b in range(2)]
    kc = [attn_pool.tile([128, T], BF16, name=f"kc{b}") for b in range(2)]
    vc = [attn_pool.tile([128, T], BF16, name=f"vc{b}") for b in range(2)]
    # vtb: [k-token, w, m, h, 36] fp8; col 32 of each 36-block is the ones col
    vtb = [attn_pool.tile([TK, NW, 4, NH, 36], F8, name=f"vtb{b}") for b in range(2)]
    lepe_sb = [attn_pool.tile([128, T], BF16, name=f"lepe{b}") for b in range(2)]
    att = attn_pool.tile([128, 2, T], F8, name="att")
    # double-buffered exp output: [k-token, m, h, q]
    etb = [attn_pool.tile([TK, 4, NH, L], F8, name=f"et{z}") for z in range(2)]
    dg = [attn_pool.tile([128, 9, 128], BF16, name=f"dg{b}") for b in range(2)]

    # zero-init (Pool, overlapped with loads): vtb + et m3 tail rows
    for b in range(2):
        nc.gpsimd.memset(vtb[b], 0.0)
    for z in range(2):
        nc.gpsimd.memset(etb[z][:, 3, :, :], 0.0)
    # LePE diag matrices (Pool, overlapped with loads)
    for bi in range(2):
        for tx in (-1, 0, 1):
            for ty in (-1, 0, 1):
                wi = ((ty + 1) * 3 + (tx + 1)) if bi == 0 else ((tx + 1) * 3 + (ty + 1))
                nc.gpsimd.affine_select(
                    out=dg[bi][:, wi, :],
                    in_=lwb[:, bi, wi:wi + 1].broadcast_to([128, 128]),
                    compare_op=OP.is_equal, fill=0.0, base=0,
                    pattern=[[-1, 128]], channel_multiplier=1)

    # Optional in-NEFF repetition loop for wall-clock timing (BASS_KERNEL_ITERS>1)
    import os as _os
    _iters = int(_os.environ.get("BASS_KERNEL_ITERS", "1"))
    loop_cm = tc.For_i(0, _iters, 1) if _iters > 1 else None
    if loop_cm is not None:
        ctx.enter_context(loop_cm)

    # ---------------- phase B+C: LN1 pipelined with QKV ----------------
    qc0w = qc[0].rearrange("p (w h) -> p h w", w=WW)
    kc0w = kc[0].rearrange("p (w h) -> p h w", w=WW)
    vc0w = vc[0].rearrange("p (w h) -> p h w", w=WW)
    chunks = [("k0", 0, 256, nc.scalar), ("k1", 1, 384, nc.vector),
              ("q0", 0, 0, nc.scalar), ("q1", 1, 128, nc.scalar),
              ("v0", 0, 512, nc.scalar), ("v1", 1, 640, nc.vector)]
    wmaj = {"q0": qc0w, "k0": kc0w, "v0": vc0w}
    hmaj = {"q1": qc[1], "k1": kc[1], "v1": vc[1]}

    with tc.tile_pool(name="lnp1", bufs=8) as lnp, \
         tc.tile_pool(name="tpp1", bufs=2, space="PSUM") as tpp, \
         tc.tile_pool(name="qkvps", bufs=3, space="PSUM") as qkvps:

        def emit_qkv_t2(t2):
            for nm, bi, c0, eng in chunks:
                pt = qkvps.tile([128, 2, 512], F32, name="qkvt", tag="qkvt")
                for half in range(2):
                    t = 2 * t2 + half
                    nc.tensor.matmul(pt[:, half, 0:L],
                                     wqkv8[:, :, c0:c0 + 128],
                                     ln1[:, :, L * t:L * (t + 1)],
                                     start=True, stop=True,
                                     perf_mode=PM.DoubleRow)
                t0 = 2 * t2
                if nm in wmaj:
                    dstw = wmaj[nm][:, 7 * t0:7 * t0 + 14, :].rearrange(
                        "p (a h) w -> p a h w", a=2)
                    s = pt[:, :, 0:L].rearrange("p a (x y) -> p a x y", x=7)
                    if eng is nc.scalar:
                        eng.copy(dstw, s)
                    else:
                        eng.tensor_copy(dstw, s)
                else:
                    dsth = hmaj[nm][:, L * t0:L * (t0 + 2)].rearrange(
                        "p (a x) -> p a x", a=2)
                    if eng is nc.scalar:
                        eng.copy(dsth, pt[:, :, 0:L])
                    else:
                        eng.tensor_copy(dsth, pt[:, :, 0:L])

        qkv_done = 0
        for g in range(7):
            i0 = 4 * g
            for i in range(i0, i0 + 4):
                st = lnp.tile([TK, 6], F32, name="bnst", tag="bnst")
                nc.vector.bn_stats(st, xtok[:, i, :])
                nc.vector.bn_aggr(mv1[:, i, :], st)
            nc.scalar.activation(std1[:, i0:i0 + 4], mv1[:, i0:i0 + 4, 1],
                                 AF.Sqrt, bias=eps_t)
            nc.vector.reciprocal(rstd1[:, i0:i0 + 4], std1[:, i0:i0 + 4])
            lnt = []
            for j in range(4):
                i = i0 + j
                lt = lnp.tile([TK, C], BF16, name="lnt", tag="lnt")
                nc.vector.tensor_scalar(
                    out=lt, in0=xtok[:, i, :],
                    scalar1=mv1[:, i, 0:1], scalar2=rstd1[:, i:i + 1],
                    op0=OP.subtract, op1=OP.mult)
                lnt.append(lt)
            for c in range(2):
                tp = tpp.tile([128, 4 * TK], BF16, name="lntp", tag="lntp")
                for j in range(4):
                    nc.tensor.transpose(tp[:, TK * j:TK * (j + 1)],
                                        lnt[j][:, 128 * c:128 * (c + 1)],
                                        id_b[0:TK, 0:TK])
                nc.scalar.activation(ln1[:, c, 4 * TK * g:4 * TK * (g + 1)], tp,
                                     AF.Identity, bias=b1s[:, c:c + 1],
                                     scale=g1s[:, c:c + 1])
            while qkv_done < 4 and 784 * (qkv_done + 1) <= 448 * (g + 1):
                emit_qkv_t2(qkv_done)
                qkv_done += 1

    # ---------------- phase D: windowed attention ----------------
    # PSUM: sp tag = [128, 2, 512] f32 (2 banks) x2 bufs; aux tag = 1-bank
    # x4 bufs holding lp / otA / otB / ptp / prt. exp runs per (m, head-pair)
    # so S double-buffering never waits on the window tail.
    att0w = att[:, 0, :].rearrange("p (w h) -> p h w", w=WW)
    taps = [(0, 0)] + [(tx, ty) for tx in (-1, 0, 1) for ty in (-1, 0, 1)
                       if (tx, ty) != (0, 0)]

    def emit_proj(i2, pool):
        pt = pool.tile([TK, 2, C], F32, name="prt", tag="aux", bufs=3)
        for half in range(2):
            i = 2 * i2 + half
            nc.tensor.matmul(pt[:, half, :], ones1, brow_sb[:, 0, :],
                             start=(half == 0), stop=False,
                             skip_group_check=True)
            nc.tensor.matmul(pt[:, half, :],
                             att[:, :, TK * i:TK * (i + 1)],
                             wproj8,
                             start=False, stop=(half == 1),
                             perf_mode=PM.DoubleRow, skip_group_check=True)
        nc.vector.scalar_tensor_tensor(
            out=x2res[:, 2 * i2:2 * i2 + 2, :], in0=pt, scalar=1.0,
            in1=xtok[:, 2 * i2:2 * i2 + 2, :], op0=OP.mult, op1=OP.add)
        # LN2 stats for these two token tiles (DVE, no PSUM)
        for i in (2 * i2, 2 * i2 + 1):
            st = onp.tile([TK, 6], F32, name="bnst2", tag="bnst2")
            nc.vector.bn_stats(st, x2res[:, i, :])
            nc.vector.bn_aggr(mv2[:, i, :], st)

    proj_done = 0
    with tc.tile_pool(name="dps", bufs=2, space="PSUM") as dps, \
         tc.tile_pool(name="onp", bufs=3) as onp, \
         tc.tile_pool(name="rqp", bufs=3) as rqp:
        def emit_vtb(bj, wj):
            vt = dps.tile([TK, 4, 128], BF16, name="vtt", tag="aux", bufs=3)
            for m in range(4):
                mc = MCS[m]
                nc.tensor.transpose(
                    vt[0:mc, m, :],
                    vc[bj][:, L * wj + MCO[m]:L * wj + MCO[m] + mc],
                    id_b)
            vtv = vt.rearrange("p m (h e) -> p m h e", h=NH)
            nc.vector.tensor_copy(vtb[bj][:, wj, 0:3, :, 0:32], vtv[:, 0:3])
            nc.vector.tensor_copy(vtb[bj][0:56, wj, 3, :, 0:32], vtv[0:56, 3])
            nc.vector.memset(vtb[bj][:, wj, :, :, 32:33], 1.0)

        def emit_sexp(idx):
            bi, w = idx // NW, idx % NW
            et = etb[idx % 2]
            for m in range(4):
                mc = MCS[m]
                for hp in range(2):
                    sp = dps.tile([128, 2, 512], F32, name="spst", tag="sp")
                    for hh in range(2):
                        h = 2 * hp + hh
                        nc.tensor.matmul(
                            sp[0:mc, hh, 0:L],
                            kc[bi][32 * h:32 * (h + 1),
                                   L * w + MCO[m]:L * w + MCO[m] + mc],
                            qc[bi][32 * h:32 * (h + 1), L * w:L * (w + 1)],
                            start=True, stop=True, tile_position=(32 * h, 0))
                    nc.scalar.activation(
                        et[0:mc, m, 2 * hp:2 * hp + 2, :],
                        sp[0:mc, :, 0:L],
                        AF.Exp, scale=SM_SCALE)

        def emit_tail(idx):
            nonlocal proj_done
            bi, w = idx // NW, idx % NW
            et = etb[idx % 2]
            vcv = vc[bi][:, L * w:L * (w + 1)].rearrange("p (x y) -> p x y", x=7)
            if bi == 0:
                emit_vtb(0, w)
                emit_vtb(1, w)
            # LePE taps
            lp = dps.tile([128, 512], F32, name="lpt", tag="aux", bufs=3)
            lpv = lp[:, 0:L].rearrange("p (x y) -> p x y", x=7)
            for ti, (tx, ty) in enumerate(taps):
                wi = ((ty + 1) * 3 + (tx + 1)) if bi == 0 else ((tx + 1) * 3 + (ty + 1))
                xo0, xo1 = max(0, -tx), 7 - max(0, tx)
                yo0, yo1 = max(0, -ty), HH - max(0, ty)
                nc.tensor.matmul(
                    lpv[:, xo0:xo1, yo0:yo1], dg[bi][:, wi, :],
                    vcv[:, xo0 + tx:xo1 + tx, yo0 + ty:yo1 + ty],
                    start=(ti == 0), stop=(ti == 8))
            nc.vector.tensor_scalar(
                out=lepe_sb[bi][:, L * w:L * (w + 1)], in0=lp[:, 0:L],
                scalar1=lbs[:, bi:bi + 1], scalar2=None, op0=OP.add)
            # O^T via fp8 DoubleRow
            ots = []
            for qp in range(2):
                ot = dps.tile([TK, 2, NH, 34], F32, name="otst", tag="aux",
                              bufs=3)
                ots.append(ot)
                first = True
                for qq in range(2):
                    q = 2 * qp + qq
                    qn = MCS[q]
                    for j in range(2):
                        for h in range(NH):
                            nc.tensor.matmul(
                                ot[0:qn, qq, h, 0:33],
                                et[:, 2 * j:2 * j + 2, h, MCO[q]:MCO[q] + qn],
                                vtb[bi][:, w, 2 * j:2 * j + 2, h, 0:33],
                                start=first,
                                stop=(qq == 1 and j == 1 and h == NH - 1),
                                perf_mode=PM.DoubleRow, skip_group_check=True)
                            first = False
            ptp = dps.tile([128, NH, TK], BF16, name="ptp", tag="aux", bufs=3)
            for q in range(4):
                qn, qo = MCS[q], MCO[q]
                ot = ots[q // 2]
                rq = rqp.tile([TK, NH], F32, name="rq", tag="rq")
                nc.vector.reciprocal(rq[0:qn], ot[0:qn, q % 2, :, 32])
                on = onp.tile([TK, NH * 32], BF16, name="on", tag="on")
                onv = on.rearrange("p (a b) -> p a b", a=NH)
                nc.vector.tensor_tensor(
                    out=onv[0:qn], in0=ot[0:qn, q % 2, :, 0:32],
                    in1=rq[0:qn].unsqueeze(2).broadcast_to([qn, NH, 32]),
                    op=OP.mult)
                nc.tensor.transpose(ptp[:, q, 0:qn], on[0:qn, :],
                                    id_b[0:qn, 0:qn])
                lep = lepe_sb[bi][:, L * w + qo:L * w + qo + qn]
                if bi == 0:
                    dst = att0w[:, :, 7 * w + qo // 56:7 * w + (qo + qn) // 56]
                    nc.vector.tensor_tensor(
                        out=dst,
                        in0=ptp[:, q, 0:qn].rearrange("p (a b) -> p b a", b=HH),
                        in1=lep.rearrange("p (a b) -> p b a", b=HH),
                        op=OP.add)
                else:
                    nc.vector.tensor_tensor(
                        out=att[:, 1, L * w + qo:L * w + qo + qn],
                        in0=ptp[:, q, 0:qn], in1=lep, op=OP.add)
            if bi == 1:
                while 224 * (proj_done + 1) <= 392 * (w + 1):
                    emit_proj(proj_done, dps)
                    proj_done += 1

        emit_sexp(0)
        for idx in range(2 * NW):
            if idx + 1 < 2 * NW:
                emit_sexp(idx + 1)
            emit_tail(idx)
    # ---------------- phase E: proj + residual (token-major out) ----------------
    actx.close()

    # ---------------- phase F: LN2 + MLP ----------------
    with tc.tile_pool(name="mlp", bufs=1) as mlp:
        h_sb = mlp.tile([128, 8, T], F8, name="h_sb")
        with tc.tile_pool(name="lnp2", bufs=8) as lnp2, \
             tc.tile_pool(name="tpp2", bufs=2, space="PSUM") as tpp2:
            for gs, ge in ((0, 6), (6, 7)):
                i0 = 4 * gs
                i1 = 4 * ge
                nc.scalar.activation(std2[:, i0:i1], mv2[:, i0:i1, 1],
                                     AF.Sqrt, bias=eps_t)
                nc.vector.reciprocal(rstd2[:, i0:i1], std2[:, i0:i1])
                for g in range(gs, ge):
                    lnt = []
                    for j in range(4):
                        i = 4 * g + j
                        lt = lnp2.tile([TK, C], BF16, name="lnt2", tag="lnt2")
                        nc.vector.tensor_scalar(
                            out=lt, in0=x2res[:, i, :],
                            scalar1=mv2[:, i, 0:1], scalar2=rstd2[:, i:i + 1],
                            op0=OP.subtract, op1=OP.mult)
                        lnt.append(lt)
                    for c in range(2):
                        tp = tpp2.tile([128, 4 * TK], BF16, name="lntp2",
                                       tag="lntp2")
                        for j in range(4):
                            nc.tensor.transpose(tp[:, TK * j:TK * (j + 1)],
                                                lnt[j][:, 128 * c:128 * (c + 1)],
                                                id_b[0:TK, 0:TK])
                        nc.scalar.activation(
                            ln2[:, c, 4 * TK * g:4 * TK * (g + 1)], tp,
                            AF.Identity, bias=b2s[:, c:c + 1],
                            scale=g2s[:, c:c + 1])
        with tc.tile_pool(name="f1ps", bufs=2, space="PSUM") as f1ps, \
             tc.tile_pool(name="f2ps", bufs=2, space="PSUM") as f2ps, \
             tc.tile_pool(name="otp", bufs=4) as otp:
            fc2_done = 0
            for tp2 in range(NW // 2):
                tparts = [(2 * tp2, 2)]
                for tb, tn in tparts:
                    for m8 in range(8):
                        pt = f1ps.tile([128, 2, 512], F32, name="f1t", tag="f1t")
                        for half in range(tn):
                            t = tb + half
                            nc.tensor.matmul(pt[:, half, 0:L],
                                             wfc18[:, :, 128 * m8:128 * (m8 + 1)],
                                             ln2[:, :, L * t:L * (t + 1)],
                                             start=True, stop=True,
                                             perf_mode=PM.DoubleRow)
                        nc.scalar.activation(
                            h_sb[:, m8, L * tb:L * (tb + tn)].rearrange(
                                "p (a x) -> p a x", a=tn),
                            pt[:, 0:tn, 0:L],
                            AF.Gelu, bias=bfc1s[:, m8:m8 + 1])
                    while 224 * (fc2_done + 1) <= 392 * (tb + tn):
                        i2 = fc2_done
                        pt = f2ps.tile([TK, 2, C], F32, name="f2t", tag="f2t")
                        for half in range(2):
                            i = 2 * i2 + half
                            nc.tensor.matmul(pt[:, half, :], ones1,
                                             brow_sb[:, 1, :],
                                             start=(half == 0), stop=False,
                                             skip_group_check=True)
                            for j in range(4):
                                nc.tensor.matmul(
                                    pt[:, half, :],
                                    h_sb[:, 2 * j:2 * j + 2, TK * i:TK * (i + 1)],
                                    wfc28[:, 2 * j:2 * j + 2, :],
                                    start=False,
                                    stop=(half == 1 and j == 3),
                                    perf_mode=PM.DoubleRow,
                                    skip_group_check=True)
                        ot = otp.tile([TK, 2, C], F32, name="ot", tag="ot")
                        nc.vector.scalar_tensor_tensor(
                            out=ot, in0=pt, scalar=1.0,
                            in1=x2res[:, 2 * i2:2 * i2 + 2, :],
                            op0=OP.mult, op1=OP.add)
                        eng = nc.sync if i2 % 2 == 0 else nc.scalar
                        eng.dma_start(
                            out_d[2 * TK * i2:2 * TK * (i2 + 1), :].rearrange(
                                "(a p) c -> p a c", p=TK),
                            ot)
                        fc2_done += 1
            if False:
                while 224 * (fc2_done + 1) <= 784 * (tp2 + 1):
                    i2 = fc2_done
                    pt = f2ps.tile([TK, 2, C], F32, name="f2t", tag="f2t")
                    for half in range(2):
                        i = 2 * i2 + half
                        nc.tensor.matmul(pt[:, half, :], ones1, brow_sb[:, 1, :],
                                         start=(half == 0), stop=False,
                                         skip_group_check=True)
                        for j in range(4):
                            nc.tensor.matmul(pt[:, half, :],
                                             h_sb[:, 2 * j:2 * j + 2, TK * i:TK * (i + 1)],
                                             wfc28[:, 2 * j:2 * j + 2, :],
                                             start=False,
                                             stop=(half == 1 and j == 3),
                                             perf_mode=PM.DoubleRow,
                                             skip_group_check=True)
                    ot = otp.tile([TK, 2, C], F32, name="ot", tag="ot")
                    nc.vector.scalar_tensor_tensor(
                        out=ot, in0=pt, scalar=1.0,
                        in1=x2res[:, 2 * i2:2 * i2 + 2, :],
                        op0=OP.mult, op1=OP.add)
                    eng = nc.sync if i2 % 2 == 0 else nc.scalar
                    eng.dma_start(
                        out_d[2 * TK * i2:2 * TK * (i2 + 1), :].rearrange(
                            "(a p) c -> p a c", p=TK),
                        ot)
                    fc2_done += 1


def kernel(**inputs):
    if "nc" not in _CACHE:
        _CACHE["nc"] = _build()
    nc = _CACHE["nc"]

    import ml_dtypes
    FP8 = ml_dtypes.float8_e4m3
    x = np.asarray(inputs["x"], dtype=np.float32)          # [8, 56, 56, 256]

    def pack8(w, parts):
        w = np.asarray(w, np.float32)
        return np.ascontiguousarray(
            w.reshape(parts, 128, w.shape[1]).transpose(1, 0, 2)).astype(FP8)

    lw = np.stack([np.asarray(inputs["lepe_w0"], np.float32).reshape(128, 9),
                   np.asarray(inputs["lepe_w1"], np.float32).reshape(128, 9)],
                  axis=1)
    brow = np.stack([np.asarray(inputs["b_proj"], np.float32),
                     np.asarray(inputs["b_fc2"], np.float32)])[None]
    base = {
        "w_qkv8": pack8(inputs["w_qkv"], 2),
        "w_proj8": pack8(inputs["w_proj"], 2),
        "gamma1": np.asarray(inputs["gamma1"], np.float32),
        "beta1": np.asarray(inputs["beta1"], np.float32),
        "gamma2": np.asarray(inputs["gamma2"], np.float32),
        "beta2": np.asarray(inputs["beta2"], np.float32),
        "w_fc18": pack8(inputs["w_fc1"], 2),
        "b_fc1": np.asarray(inputs["b_fc1"], np.float32),
        "w_fc28": pack8(inputs["w_fc2"], 8),
        "lwb16": lw.astype(ml_dtypes.bfloat16),
        "brow16": brow.astype(ml_dtypes.bfloat16),
        "lepe_b0": np.asarray(inputs["lepe_b0"], np.float32),
        "lepe_b1": np.asarray(inputs["lepe_b1"], np.float32),
    }
    in_maps = [{**base, "x": np.ascontiguousarray(x[i].reshape(T, C))}
               for i in range(B)]
    import os
    trace = bool(int(os.environ.get("BASS_KERNEL_TRACE", "0")))
    res = run_bass_kernel_spmd(nc, in_maps, core_ids=list(range(B)), trace=trace)
    _CACHE["last_results"] = res
    out = np.stack([res.results[i]["out"] for i in range(B)])
    return out.reshape(B, HH, WW, C)


if __name__ == "__main__":
    rng = np.random.default_rng(0)
    ins = {
        "x": rng.standard_normal((B, HH, WW, C), dtype=np.float32),
        "gamma1": np.ones(C, np.float32), "beta1": np.zeros(C, np.float32),
        "w_qkv": rng.standard_normal((C, 3 * C), dtype=np.float32) * 0.02,
        "lepe_w0": rng.standard_normal((128, 1, 3, 3), dtype=np.float32) * 0.02,
        "lepe_b0": np.zeros(128, np.float32),
        "lepe_w1": rng.standard_normal((128, 1, 3, 3), dtype=np.float32) * 0.02,
        "lepe_b1": np.zeros(128, np.float32),
        "w_proj": rng.standard_normal((C, C), dtype=np.float32) * 0.02,
        "b_proj": np.zeros(C, np.float32),
        "gamma2": np.ones(C, np.float32), "beta2": np.zeros(C, np.float32),
        "w_fc1": rng.standard_normal((C, 4 * C), dtype=np.float32) * 0.02,
        "b_fc1": np.zeros(4 * C, np.float32),
        "w_fc2": rng.standard_normal((4 * C, C), dtype=np.float32) * 0.02,
        "b_fc2": np.zeros(C, np.float32),
    }
    o = kernel(**ins)
    print("ran:", o.shape, o.dtype, float(np.abs(o).max()))
